# revision 63
# baseline (speedup 1.0000x reference)
"""CALoraLinear kernel for 8 TRN2 NeuronCores (Bass/Tile, SPMD).

Math (derived from the reference):
  orig = x @ W.T + bias
  top2 classes c1,c2 per row from pseudo_index[b, :64]
  g_j = <lora_A[c_j], x[b]>          (only rows 0..63 of lora_A are reachable)
  lora_out[b,o] = 16 * sum_c mask[b,c] * G[b,c] * lora_B[o,c]
  out = orig + lora_out + bias       (bias added twice)

Sharding: column-shard W across the 8 cores (each core owns 512 output
columns, full batch); x / lora_A / pseudo_index replicated. Host
concatenates the per-core [512, 512] blocks along the output axis.

Precision plan (rel err 1.814e-2 vs the 2e-2 gate, fixed seed,
bit-reproducible on hardware and matching the numpy model to 5 digits):
  - k-tiles 0..N8-1 (N8=10) of the main matmul run in fp8e4 DoubleRow
    mode (2 k-tiles per 216ns PE pass = 2x fp16): both x and W
    single-quantized. Shipping these blocks as fp8 also trims stream
    bytes. Error scales ~2.8e-2*sqrt(N8/32).
  - The other k-tiles stay fp16. All W is shipped pre-scaled by 256 so
    fp8 W (~N(0,0.02)) sits in e4m3's normal range and the PSUM scale
    is uniform; the output copy divides by 256.
  - The G matmul is all-fp8 DoubleRow: x8 comes from the fp8 stream
    or on-chip DVE casts; the stationary packs [a8 | a-residual*16]
    into its 128 columns (the residual rides the previously-wasted pad
    half), killing the a-quantization error.
  - fp16 output, host upcasts.

Schedule: fp16 stream first on both HWDGE rings (first k-tile split
across rings, singles while the PE ramp is hungry, then pairs — pair
chunks at the front repeatedly landed right at their need time, while
too many triggers at the back lose to the ring-slot throttle), with ps
early (the mask chain hangs off it), the a-quarters and the last pair
mid-queue, and the fp8 pair-chunks at the back.
Dependency-precision matters: one tile per fp8 pair and per a-quarter,
because rearranged APs coarsen region tracking to whole-tile and a
reader then waits for the LAST DMA into the tile. PE: warm-up ramp,
fp16 mains k8-31 (4 mask transposes slotted mid-stream; the top-2
mask is computed in [b,c] orientation and PE-transposed — a DMA
transpose bounce crawls through the saturated queues), one closing
fp8-DR block (all 16 G pairs, then the k0-7 DR mains, whose ~3.5us
cover the DVE/Pool ht chain), then the 4 lora matmuls with overlapped
scaled copies + output DMAs. Mode switches fp16<->fp8DR throttle the
PE ~20% when frequent — and any PE idle gap resets the DVFS ramp
(1.2GHz for the next ~3us) — so the fp8 work is one contiguous block
placed where its inputs have long landed. Exec time counts until the
last DMA (+ ~3us teardown), so the drain is critical path.
"""

import os
import sys

for _p in ("/opt/trn_rl_repo",):
    if _p not in sys.path:
        sys.path.insert(0, _p)

import numpy as np
import ml_dtypes

import concourse.bass as bass
import concourse.bacc as bacc
import concourse.mybir as mybir
from concourse.tile import TileContext, add_dep_helper
from concourse.bass_utils import run_bass_kernel_spmd
from concourse.masks import make_identity


def _ensure_ntff_hook_module():
    """run_bass_kernel_spmd(trace=True) imports antenv.axon_hooks, which the
    agent image's antenv package lacks. Provide it (and register the real
    ctypes NTFF hook when available) so a tracing caller doesn't crash."""
    import types

    try:
        import antenv
    except ImportError:
        return
    if getattr(antenv, "axon_hooks", None) is not None:
        return
    mod = types.ModuleType("antenv.axon_hooks")
    state = {"hook": None}
    mod.set_axon_ntff_profile_hook = lambda h: state.__setitem__("hook", h)
    mod.get_axon_ntff_profile_hook = lambda: state["hook"]
    sys.modules["antenv.axon_hooks"] = mod
    antenv.axon_hooks = mod
    try:
        from trn_agent_boot.trn_boot import _ntff_profile_via_ctypes

        mod.set_axon_ntff_profile_hook(
            _ntff_profile_via_ctypes("/opt/axon/libaxon_pjrt.so")
        )
    except Exception:
        pass


_ensure_ntff_hook_module()

B, IN, OUT = 512, 4096, 4096
NUM_CLASS, RANK = 64, 8
NCORES = 8
OUT_L = OUT // NCORES  # 512
P = 128
KT = IN // P           # 32 k-tiles
BT = B // P            # 4 batch tiles
NPAIR = KT // 2        # 16 fp8 DoubleRow G passes

N8 = 10                # k-tiles 0..N8-1 run the MAIN matmul in fp8 DR
NP8 = N8 // 2          # 4 main/G DoubleRow pairs in the fp8 block
NF = KT - N8           # 24 fp16 k-tiles (k8..k31)

SA = 256.0             # host-side scale on lora_A before fp8 quantization
SR = 16.0              # extra scale on the a-residual half
SW = 256.0             # host-side scale on W (fp8 range + uniform PSUM scale)

# per-k-tile stream block layouts: [x: B][w: OUT_L]
KW8 = B + OUT_L        # 1024 fp8 elements (k0..7)
KW = B + OUT_L         # 1024 fp16 cols (k8..31)

WARM = int(os.environ.get("WARM", "38"))

# fp16 stream chunk schedule in kk-space (kk = k - N8): first fp16
# k-tile single (split across rings at trigger time), then singles
# through kk7 (finer early granularity keeps supply ahead of the PE —
# the first 512KB pair chunk repeatedly landed right at its need time,
# a ~1.8us stall), then pairs
NSGL = 8
CHUNKS16 = [(kk, 1) for kk in range(NSGL)] + [
    (kk, 2) for kk in range(NSGL, NF, 2)
]
assert NF % 2 == 0 and N8 % 2 == 0 and NSGL < NF

F32 = mybir.dt.float32
F32R = mybir.dt.float32r
F16 = mybir.dt.float16
F8 = mybir.dt.float8e4
X = mybir.AxisListType.X
DR = mybir.MatmulPerfMode.DoubleRow

_cache = {}
# test.py reads this after a traced run for HW exec time
last_results = None


def _build():
    key = "nc"
    if key in _cache:
        return _cache[key]
    nc = bacc.Bacc(
        bass.get_trn_type() or "TRN2",
        target_bir_lowering=False,
        debug=False,
        num_devices=NCORES,
    )

    xw8 = nc.dram_tensor("xw8", [N8 * P * KW8], F8, kind="ExternalInput")
    xw = nc.dram_tensor("xw", [NF * P * KW], F16, kind="ExternalInput")
    af_d = nc.dram_tensor("af", [P, KT * P], F8, kind="ExternalInput")
    ps_d = nc.dram_tensor("ps", [P, BT * NUM_CLASS], F32, kind="ExternalInput")
    bS_d = nc.dram_tensor("bS", [NUM_CLASS + 1, OUT_L], F32R, kind="ExternalInput")
    out = nc.dram_tensor("out", [B, OUT_L], F16, kind="ExternalOutput")

    with TileContext(nc) as tc:
        with (
            tc.tile_pool(name="xwp", bufs=1) as xwpool,
            tc.tile_pool(name="sml", bufs=1) as spool,
            tc.tile_pool(name="tl", bufs=1) as tpool,
            tc.tile_pool(name="op", bufs=1) as opool,
            tc.tile_pool(name="ps", bufs=1, space="PSUM") as ppool,
        ):
            # ---- resident stream buffers. The fp8 block uses one tile per
            # DoubleRow pair so the PE's dependency is on exactly the two
            # k-tiles it reads (coarser tracking held the first DR main
            # until the whole block landed). ----
            xw8p = [
                xwpool.tile([P, 2 * KW8], F8, name=f"xw8p{p}")
                for p in range(NP8)
            ]
            xw_sb = xwpool.tile([P, NF * KW], F16)
            # one tile PER a-quarter: a single a tile gave the G passes a
            # whole-tile dependency (rearranged APs defeat region tracking),
            # so the first G pass waited for the LAST quarter's DMA — a
            # ~5us PE stall that also reset the p-state ramp
            QW = KT * P // 4  # 1024 fp8 cols per a-quarter (8 k-tiles)
            a_qsb = [
                spool.tile([P, QW], F8, name=f"aq{q}") for q in range(4)
            ]

            dma_done = {}
            # Ring trigger order. fp16 stream first (k8 split across both
            # rings for the earliest PE start, k9 single, then pairs — each
            # ring only runs a few transfers concurrently, so fewer/larger
            # triggers finish the stream tail sooner). The a-quarters and
            # the fp8-pair chunks slot in where the PE's closing fp8 block
            # will need them.
            events = []

            def ev_fp16(k0, n):
                kk = k0 - N8
                src = xw[kk * P * KW : (kk + n) * P * KW].rearrange(
                    "(p d) -> p d", p=P
                )
                dst = xw_sb[:, kk * KW : (kk + n) * KW]

                def go(eng, src=src, dst=dst, k0=k0):
                    dma_done[k0] = eng.dma_start(out=dst, in_=src)
                return go

            def ev_a(q):
                def go(eng, q=q):
                    eng.dma_start(
                        out=a_qsb[q], in_=af_d[:, q * QW : (q + 1) * QW]
                    )
                return go

            def ev_fp8(p):
                src = xw8[2 * p * P * KW8 : (2 * p + 2) * P * KW8].rearrange(
                    "(p d) -> p d", p=P
                )

                def go(eng, src=src, p=p):
                    eng.dma_start(out=xw8p[p], in_=src)
                return go

            # k8 halves on both rings first
            src8 = xw[0 : P * KW].rearrange("(p d) -> p d", p=P)
            nc.sync.dma_start(
                out=xw_sb[:, 0 : KW // 2], in_=src8[:, 0 : KW // 2]
            )
            dma_done[N8] = nc.scalar.dma_start(
                out=xw_sb[:, KW // 2 : KW], in_=src8[:, KW // 2 :]
            )
            # ps early on a ring: the mask chain (DVE reduces -> maskb ->
            # PE transposes) starts from it, and on the gated SWDGE path it
            # crawled through the saturated queues, stalling the PE
            # transposes ~3us
            ps_sb = spool.tile([P, BT * NUM_CLASS], F32)
            nc.sync.dma_start(out=ps_sb, in_=ps_d[:, :])
            # singles k(N8+1..N8+3), then pairs; the LAST pair (k30/31)
            # rides mid-queue — the PE needs it right after k29 (the finale
            # mains run before the fp8 block), and at the back of the queue
            # it landed ~30us, stalling the PE and resetting the p-state
            # for the whole fp8 block. a-quarters and fp8-pair chunks slot
            # in ahead of where the closing fp8 block needs them.
            for k0 in range(N8 + 1, N8 + NSGL):
                events.append(ev_fp16(k0, 1))
            pairs = list(range(N8 + NSGL, KT, 2))
            last = pairs.pop()                  # (30,31)
            mid = len(pairs) // 2
            for i, k0 in enumerate(pairs):
                events.append(ev_fp16(k0, 2))
                if i == mid:
                    events.append(ev_a(1))
                if i == mid + 2:
                    events.append(ev_a(2))
                if i == len(pairs) - 2:
                    events.append(ev_fp16(last, 2))
            events.append(ev_a(3))
            for p in range(NP8):
                events.append(ev_fp8(p))
            events.append(ev_a(0))
            for i, go in enumerate(events):
                go(nc.sync if i % 2 == 0 else nc.scalar)

            # ---- bS on the gpsimd SWDGE queue, gated late: it is not
            # needed until the lora tail (~37us) and ungated it competes
            # with the stream for DMA bandwidth ----
            bS_sb = spool.tile([NUM_CLASS + 1, OUT_L], F32R)
            bS_dma = nc.gpsimd.dma_start(out=bS_sb, in_=bS_d[:, :])
            add_dep_helper(
                bS_dma.ins, dma_done[16].ins,
                reason="bS yields early DMA bandwidth to the stream",
            )
            # identity for the PE mask transposes (Pool, dep-free, early)
            ident = spool.tile([P, P], F32)
            make_identity(nc, ident)

            # ---- PE warm-up: ramp the p-state during the DMA lead-in ----
            warm_src = spool.tile([P, P], F16)
            nc.vector.memset(warm_src, 0.0)
            warm_ps = ppool.tile([P, P], F32, tag="warm", name="warm_ps")
            for w in range(WARM):
                nc.tensor.matmul(
                    warm_ps, lhsT=warm_src, rhs=warm_src, start=True, stop=True
                )

            def blk8(p):
                # fp8 stream pair p as [P, 2, KW8]
                return xw8p[p].rearrange("q (two d) -> q two d", two=2)

            def xk(kk):
                return xw_sb[:, kk * KW : kk * KW + B]

            def wk(kk):
                return xw_sb[:, kk * KW + B : kk * KW + B + OUT_L]

            # ---- on-chip fp16 -> fp8 casts of x (k8..31) for the DoubleRow
            # G matmul, all on DVE (~400ns each; Pool casts cost ~1.85us).
            # The mask work is interleaved at the points where its inputs
            # land so nothing queues behind the full cast stream. ----
            x8_sb = spool.tile([P, NF * B], F8)

            def cast_x8(kk):
                nc.vector.tensor_copy(
                    out=x8_sb[:, kk * B : (kk + 1) * B], in_=xk(kk)
                )

            for kk in range(2):
                cast_x8(kk)

            # ---- top-2 threshold per batch row (free-axis reduces: DVE) ----
            m2col = spool.tile([P, BT], F32)
            for bt in range(BT):
                pt = ps_sb[:, bt * NUM_CLASS : (bt + 1) * NUM_CLASS]
                m1 = spool.tile([P, 1], F32, tag=f"m1_{bt}")
                nc.vector.reduce_max(out=m1, in_=pt, axis=X)
                negmask = spool.tile([P, NUM_CLASS], F32, tag=f"nm_{bt}")
                # (pt >= m1) * -1e30  -> additive mask that kills the max
                nc.vector.tensor_scalar(
                    out=negmask,
                    in0=pt,
                    scalar1=m1,
                    scalar2=-1.0e30,
                    op0=mybir.AluOpType.is_ge,
                    op1=mybir.AluOpType.mult,
                )
                p2 = spool.tile([P, NUM_CLASS], F32, tag=f"p2_{bt}")
                nc.vector.tensor_tensor(
                    out=p2, in0=pt, in1=negmask, op=mybir.AluOpType.add
                )
                nc.vector.reduce_max(out=m2col[:, bt : bt + 1], in_=p2, axis=X)

            # mask in [b, c] orientation: maskb[b, c] = ps[b, c] >= m2[b].
            # (The old path transposed m2 via a DRAM bounce + broadcast DMA;
            # those 4-byte-descriptor transfers crawled through the
            # stream-saturated queues and delivered maskT at ~42us,
            # stalling the lora tail ~4us. The PE transposes maskb instead.)
            maskb = tpool.tile([P, BT * NUM_CLASS], F32)
            for bt in range(BT):
                nc.vector.tensor_scalar(
                    out=maskb[:, bt * NUM_CLASS : (bt + 1) * NUM_CLASS],
                    in0=ps_sb[:, bt * NUM_CLASS : (bt + 1) * NUM_CLASS],
                    scalar1=m2col[:, bt : bt + 1],
                    scalar2=None,
                    op0=mybir.AluOpType.is_ge,
                )

            for kk in range(2, NF):
                cast_x8(kk)

            # maskT: PSUM->SBUF copy of the PE-transposed mask; the copy is
            # EMITTED inside the PE loop (after the transposes) so the
            # framework orders it after its writer
            mask_ps = ppool.tile([NUM_CLASS, B], F32, tag="mt", name="mask_ps")
            maskT = tpool.tile([NUM_CLASS, B], F32)
            maskT16 = tpool.tile([NUM_CLASS, B], F32)

            # ht rows: 0..63 filled at the end; row 64 = ones (written early,
            # on Pool: 0*x + 1 from any [1, B] source already in SBUF)
            ht = tpool.tile([NUM_CLASS + 1, B], F32R)
            nc.gpsimd.tensor_scalar(
                out=ht[NUM_CLASS : NUM_CLASS + 1, :],
                in0=xw_sb[0:1, 0:B],
                scalar1=0.0,
                scalar2=1.0,
                op0=mybir.AluOpType.mult,
                op1=mybir.AluOpType.add,
            )

            # ---- PSUM accumulators ----
            mps = [
                ppool.tile([P, OUT_L], F32, tag=f"main{bt}", name=f"main{bt}")
                for bt in range(BT)
            ]
            # G accumulator: partitions 0:64 = sum x8*a8, 64:128 = the
            # a-residual half (both halves of the packed DoubleRow stationary)
            gt_ps = ppool.tile([P, B], F32, tag="gt", name="gt_ps")

            def main_mm8(p, bt):
                b = blk8(p)
                nc.tensor.matmul(
                    mps[bt],
                    lhsT=b[:, :, bt * P : (bt + 1) * P],
                    rhs=b[:, :, B : B + OUT_L],
                    start=False,
                    stop=False,
                    perf_mode=DR,
                )

            def main_mm(kk, bt):
                nc.tensor.matmul(
                    mps[bt],
                    lhsT=xk(kk)[:, bt * P : (bt + 1) * P],
                    rhs=wk(kk),
                    start=(kk == 0),
                    stop=False,
                )

            def g_dr(j, start, stop):
                jj = j % 4
                a_ap = a_qsb[j // 4][:, 2 * jj * P : (2 * jj + 2) * P].rearrange(
                    "q (two m) -> q two m", two=2
                )
                if j < NP8:
                    x_ap = blk8(j)[:, :, 0:B]
                else:
                    kk = 2 * j - N8
                    x_ap = x8_sb[:, kk * B : (kk + 2) * B].rearrange(
                        "q (two n) -> q two n", two=2
                    )
                nc.tensor.matmul(
                    gt_ps,
                    lhsT=a_ap,
                    rhs=x_ap,
                    start=start,
                    stop=stop,
                    perf_mode=DR,
                )

            # ---- PE main stream ----
            # fp16 mains k8..k29 first: by the time the closing fp8 block
            # runs, the clock is fully ramped (an fp8/fp16 alternation or an
            # early idle gap left the DoubleRow passes at the 1.2GHz
            # p-state), the x8 casts and the fp8 stream chunks have long
            # landed, and the mode switch count is down to one in, one out.
            # The 4 mask transposes slot in mid-stream (maskb ready ~15us,
            # and the supply-tight stream gets a breather).
            for kk in range(NF - 2):
                for bt in range(BT):
                    main_mm(kk, bt)
                if kk == 8:
                    for bt in range(BT):
                        nc.tensor.transpose(
                            out=mask_ps[:, bt * P : (bt + 1) * P],
                            in_=maskb[:, bt * NUM_CLASS : (bt + 1) * NUM_CLASS],
                            identity=ident[:, :],
                        )
                    nc.vector.tensor_copy(out=maskT, in_=mask_ps[:, :])
                    # maskT/16 for the a-residual G half, off the tail path
                    nc.vector.tensor_scalar(
                        out=maskT16, in0=maskT, scalar1=1.0 / SR,
                        scalar2=None, op0=mybir.AluOpType.mult,
                    )
            # k30/k31 mains BEFORE the closing fp8 block: PSUM accumulation
            # order is commutative, and this leaves only the 4 lora matmuls
            # + drains after the block (~1.7us less serialized tail)
            for kk in (NF - 2, NF - 1):
                for bt in range(BT):
                    main_mm(kk, bt)
            # closing fp8-DR block: all 16 G pairs (4..15 first — their
            # casts finish earliest; 0..3 last carry the group stop so the
            # DVE lora chain unblocks right here), then the k0-7 DR mains
            # (whose 3.5us cover the DVE/Pool ht chain)
            for j in range(NP8, NPAIR):
                g_dr(j, start=(j == NP8), stop=False)
            for j in range(NP8):
                g_dr(j, start=False, stop=(j == NP8 - 1))
            for p in range(NP8):
                for bt in range(BT):
                    main_mm8(p, bt)

            # ht[0:64] = G8*maskT + Gr*(maskT/16), column-halved. Only DVE
            # (and ACT) may read PSUM, and each op may read at most one PSUM
            # operand: DVE does the four PSUM-read multiplies (half 0 first
            # so the bt0/bt1 lora matmuls unblock early); Pool does the
            # SBUF-only adds.
            t_sb = tpool.tile([NUM_CLASS, B], F32)
            t2_sb = tpool.tile([NUM_CLASS, B], F32)
            for half in (0, 1):
                cs = slice(half * (B // 2), (half + 1) * (B // 2))
                nc.vector.tensor_tensor(
                    out=t_sb[:, cs],
                    in0=gt_ps[0:NUM_CLASS, cs],
                    in1=maskT[:, cs],
                    op=mybir.AluOpType.mult,
                )
                nc.vector.tensor_tensor(
                    out=t2_sb[:, cs],
                    in0=gt_ps[NUM_CLASS : 2 * NUM_CLASS, cs],
                    in1=maskT16[:, cs],
                    op=mybir.AluOpType.mult,
                )
                nc.gpsimd.tensor_tensor(
                    out=ht[0:NUM_CLASS, cs],
                    in0=t_sb[:, cs],
                    in1=t2_sb[:, cs],
                    op=mybir.AluOpType.add,
                )

            # ---- finale: per bt: last two main k, lora tail matmul, scaled
            # copy (divides out the W scale), DMA out — interleaved so each
            # tile's output drain overlaps the remaining compute ----
            o_all = opool.tile([P, BT * OUT_L], F16)
            dma_eng = [nc.sync, nc.scalar, nc.sync, nc.scalar]

            def lora_mm(bt):
                nc.tensor.matmul(
                    mps[bt],
                    lhsT=ht[:, bt * P : (bt + 1) * P],
                    rhs=bS_sb,
                    start=False,
                    stop=True,
                )

            def copy_half(bt, half, eng):
                cs = slice(bt * OUT_L + half * (OUT_L // 2),
                           bt * OUT_L + (half + 1) * (OUT_L // 2))
                ps_cs = slice(half * (OUT_L // 2), (half + 1) * (OUT_L // 2))
                if eng is nc.vector:
                    nc.vector.tensor_scalar(
                        out=o_all[:, cs], in0=mps[bt][:, ps_cs],
                        scalar1=1.0 / SW, scalar2=None,
                        op0=mybir.AluOpType.mult,
                    )
                else:
                    nc.scalar.activation(
                        out=o_all[:, cs], in_=mps[bt][:, ps_cs],
                        func=mybir.ActivationFunctionType.Copy,
                        scale=1.0 / SW,
                    )

            def drain(bt, split=False):
                # scaled cast (divides out the W scale) then DMA out; the
                # last tile splits across DVE+ACT so its pipeline is shorter
                osl = o_all[:, bt * OUT_L : (bt + 1) * OUT_L]
                if split:
                    copy_half(bt, 0, nc.vector)
                    copy_half(bt, 1, nc.scalar)
                    dma_eng[bt].dma_start(
                        out=out[bt * P : (bt + 1) * P, 0 : OUT_L // 2],
                        in_=o_all[:, bt * OUT_L : bt * OUT_L + OUT_L // 2],
                    )
                    dma_eng[bt - 1].dma_start(
                        out=out[bt * P : (bt + 1) * P, OUT_L // 2 :],
                        in_=o_all[:, bt * OUT_L + OUT_L // 2 : (bt + 1) * OUT_L],
                    )
                    return
                copy_half(bt, 0, nc.vector if bt % 2 == 0 else nc.scalar)
                copy_half(bt, 1, nc.vector if bt % 2 == 0 else nc.scalar)
                dma_eng[bt].dma_start(out=out[bt * P : (bt + 1) * P, :], in_=osl)

            # loras as one f32r block after all DR mains (a lora between DR
            # mains both stops its PSUM group too early and re-triggers the
            # mode-alternation clock penalty); each tile's drain overlaps
            # the remaining loras
            for bt in range(BT):
                lora_mm(bt)
                drain(bt)

    nc.finalize()
    _cache[key] = nc
    return nc


def _pack_inputs(x, pseudo_index, weight, bias, lora_A, lora_B):
    """Build the per-core fp8/fp16 streams + replicated small inputs."""
    x16 = x.astype(np.float16)
    xT = np.ascontiguousarray(x16.T)                 # [IN, B] fp16
    x3 = xT.reshape(KT, P, B)
    x3_8 = x3.astype(ml_dtypes.float8_e4m3)          # fp8(fp16(x))

    # fp8 a-stream: per k-tile [a8 (64) | a-residual (64)] columns
    aT = lora_A[:NUM_CLASS].T * SA                   # [IN, 64] scaled
    a8 = aT.astype(ml_dtypes.float8_e4m3)
    ar = ((aT - a8.astype(np.float64)) * SR).astype(ml_dtypes.float8_e4m3)
    a3 = np.concatenate(
        [a8.reshape(KT, P, NUM_CLASS), ar.reshape(KT, P, NUM_CLASS)], axis=2
    )                                                # [KT, P, 128]
    af = np.ascontiguousarray(a3.transpose(1, 0, 2).reshape(P, KT * P))

    ps = np.ascontiguousarray(
        pseudo_index.reshape(BT, P, NUM_CLASS)
        .transpose(1, 0, 2)
        .reshape(P, BT * NUM_CLASS)
    )
    in_maps = []
    for i in range(NCORES):
        o0 = i * OUT_L
        wTs = weight[o0 : o0 + OUT_L].T * SW         # [IN, OUT_L] scaled
        w3_16 = wTs.astype(np.float16).reshape(KT, P, OUT_L)
        # fp8 stream (k0..7): [x8 | w8], packed as per-pair [P, 2*KW8] blocks
        k8b = np.empty((N8, P, KW8), dtype=ml_dtypes.float8_e4m3)
        k8b[:, :, 0:B] = x3_8[:N8]
        k8b[:, :, B:] = wTs.reshape(KT, P, OUT_L)[:N8].astype(
            ml_dtypes.float8_e4m3
        )
        xw8i = np.empty(N8 * P * KW8, dtype=ml_dtypes.float8_e4m3)
        for p in range(NP8):
            xw8i[2 * p * P * KW8 : (2 * p + 2) * P * KW8] = (
                k8b[2 * p : 2 * p + 2].transpose(1, 0, 2).reshape(-1)
            )
        # fp16 stream (k8..31): [x16 | w16], packed per CHUNKS16 blocks
        k3 = np.empty((NF, P, KW), dtype=np.float16)
        k3[:, :, 0:B] = x3[N8:]
        k3[:, :, B:] = w3_16[N8:]
        xwi = np.empty(NF * P * KW, dtype=np.float16)
        for kk0, n in CHUNKS16:
            xwi[kk0 * P * KW : (kk0 + n) * P * KW] = (
                k3[kk0 : kk0 + n].transpose(1, 0, 2).reshape(-1)
            )
        bS = np.empty((NUM_CLASS + 1, OUT_L), dtype=np.float32)
        # lora scaling 16 * (SW / SA) = 16; double bias * SW = 512
        bS[:NUM_CLASS] = (16.0 * SW / SA) * lora_B[o0 : o0 + OUT_L, :NUM_CLASS].T
        bS[NUM_CLASS] = 2.0 * SW * bias[o0 : o0 + OUT_L]
        in_maps.append({
            "xw8": xw8i, "xw": xwi,
            "af": af, "ps": ps, "bS": bS,
        })
    return in_maps


def kernel(x, pseudo_index, weight, bias, lora_A, lora_B):
    global last_results
    x = np.ascontiguousarray(np.asarray(x, dtype=np.float32))
    pseudo_index = np.ascontiguousarray(np.asarray(pseudo_index, dtype=np.float32))
    weight = np.asarray(weight, dtype=np.float32)
    bias = np.asarray(bias, dtype=np.float32)
    lora_A = np.asarray(lora_A, dtype=np.float32)
    lora_B = np.asarray(lora_B, dtype=np.float32)

    nc = _build()
    in_maps = _pack_inputs(x, pseudo_index, weight, bias, lora_A, lora_B)
    res = run_bass_kernel_spmd(nc, in_maps, list(range(NCORES)))
    last_results = res
    return np.hstack(
        [res.results[i]["out"] for i in range(NCORES)]
    ).astype(np.float32)


# revision 65
# speedup vs baseline: 1.0824x; 1.0824x over previous
"""CALoraLinear kernel for 8 TRN2 NeuronCores (Bass/Tile, SPMD).

Math (derived from the reference):
  orig = x @ W.T + bias
  top2 classes c1,c2 per row from pseudo_index[b, :64]
  g_j = <lora_A[c_j], x[b]>          (only rows 0..63 of lora_A are reachable)
  lora_out[b,o] = 16 * sum_c mask[b,c] * G[b,c] * lora_B[o,c]
  out = orig + lora_out + bias       (bias added twice)

Sharding: column-shard W across the 8 cores (each core owns 512 output
columns, full batch); x / lora_A / pseudo_index replicated. Host
concatenates the per-core [512, 512] blocks along the output axis.

Precision plan (rel err 1.814e-2 vs the 2e-2 gate, fixed seed,
bit-reproducible on hardware and matching the numpy model to 5 digits):
  - k-tiles 0..N8-1 (N8=10) of the main matmul run in fp8e4 DoubleRow
    mode (2 k-tiles per 216ns PE pass = 2x fp16): both x and W
    single-quantized. Shipping these blocks as fp8 also trims stream
    bytes. Error scales ~2.8e-2*sqrt(N8/32).
  - The other k-tiles stay fp16. All W is shipped pre-scaled by 256 so
    fp8 W (~N(0,0.02)) sits in e4m3's normal range and the PSUM scale
    is uniform; the output copy divides by 256.
  - The G matmul is all-fp8 DoubleRow: x8 comes from the fp8 stream
    or on-chip DVE casts; the stationary packs [a8 | a-residual*16]
    into its 128 columns (the residual rides the previously-wasted pad
    half), killing the a-quantization error.
  - fp16 output, host upcasts.

Schedule: fp16 stream first on both HWDGE rings (first k-tile split
across rings, singles while the PE ramp is hungry, then pairs — pair
chunks at the front repeatedly landed right at their need time, while
too many triggers at the back lose to the ring-slot throttle), with ps
early (the mask chain hangs off it), the a-quarters and the last pair
mid-queue, and the fp8 pair-chunks at the back.
Dependency-precision matters: one tile per fp8 pair and per a-quarter,
because rearranged APs coarsen region tracking to whole-tile and a
reader then waits for the LAST DMA into the tile. PE: warm-up ramp,
fp16 mains k8-31 (4 mask transposes slotted mid-stream; the top-2
mask is computed in [b,c] orientation and PE-transposed — a DMA
transpose bounce crawls through the saturated queues), one closing
fp8-DR block (all 16 G pairs, then the k0-7 DR mains, whose ~3.5us
cover the DVE/Pool ht chain), then the 4 lora matmuls with overlapped
scaled copies + output DMAs. Mode switches fp16<->fp8DR throttle the
PE ~20% when frequent — and any PE idle gap resets the DVFS ramp
(1.2GHz for the next ~3us) — so the fp8 work is one contiguous block
placed where its inputs have long landed. Exec time counts until the
last DMA (+ ~3us teardown), so the drain is critical path.
"""

import os
import sys

for _p in ("/opt/trn_rl_repo",):
    if _p not in sys.path:
        sys.path.insert(0, _p)

import numpy as np
import ml_dtypes

import concourse.bass as bass
import concourse.bacc as bacc
import concourse.mybir as mybir
from concourse.tile import TileContext, add_dep_helper
from concourse.bass_utils import run_bass_kernel_spmd
from concourse.masks import make_identity


def _ensure_ntff_hook_module():
    """run_bass_kernel_spmd(trace=True) imports antenv.axon_hooks, which the
    agent image's antenv package lacks. Provide it (and register the real
    ctypes NTFF hook when available) so a tracing caller doesn't crash."""
    import types

    try:
        import antenv
    except ImportError:
        return
    if getattr(antenv, "axon_hooks", None) is not None:
        return
    mod = types.ModuleType("antenv.axon_hooks")
    state = {"hook": None}
    mod.set_axon_ntff_profile_hook = lambda h: state.__setitem__("hook", h)
    mod.get_axon_ntff_profile_hook = lambda: state["hook"]
    sys.modules["antenv.axon_hooks"] = mod
    antenv.axon_hooks = mod
    try:
        from trn_agent_boot.trn_boot import _ntff_profile_via_ctypes

        mod.set_axon_ntff_profile_hook(
            _ntff_profile_via_ctypes("/opt/axon/libaxon_pjrt.so")
        )
    except Exception:
        pass


_ensure_ntff_hook_module()

B, IN, OUT = 512, 4096, 4096
NUM_CLASS, RANK = 64, 8
NCORES = 8
OUT_L = OUT // NCORES  # 512
P = 128
KT = IN // P           # 32 k-tiles
BT = B // P            # 4 batch tiles
NPAIR = KT // 2        # 16 fp8 DoubleRow G passes

N8 = 10                # k-tiles 0..N8-1 run the MAIN matmul in fp8 DR
NP8 = N8 // 2          # 4 main/G DoubleRow pairs in the fp8 block
NF = KT - N8           # 24 fp16 k-tiles (k8..k31)

SA = 256.0             # host-side scale on lora_A before fp8 quantization
SR = 16.0              # extra scale on the a-residual half
SW = 256.0             # host-side scale on W (fp8 range + uniform PSUM scale)

# per-k-tile stream block layouts: [x: B][w: OUT_L]
KW8 = B + OUT_L        # 1024 fp8 elements (k0..7)
KW = B + OUT_L         # 1024 fp16 cols (k8..31)

WARM = int(os.environ.get("WARM", "38"))

# fp16 stream chunk schedule in kk-space (kk = k - N8): first fp16
# k-tile single (split across rings at trigger time), then singles
# through kk7 (finer early granularity keeps supply ahead of the PE —
# the first 512KB pair chunk repeatedly landed right at its need time,
# a ~1.8us stall), then pairs
NSGL = 12
CHUNKS16 = [(kk, 1) for kk in range(NSGL)] + [
    (kk, 2) for kk in range(NSGL, NF, 2)
]
assert NF % 2 == 0 and N8 % 2 == 0 and NSGL < NF

F32 = mybir.dt.float32
F32R = mybir.dt.float32r
F16 = mybir.dt.float16
F8 = mybir.dt.float8e4
X = mybir.AxisListType.X
DR = mybir.MatmulPerfMode.DoubleRow

_cache = {}
# test.py reads this after a traced run for HW exec time
last_results = None


def _build():
    key = "nc"
    if key in _cache:
        return _cache[key]
    nc = bacc.Bacc(
        bass.get_trn_type() or "TRN2",
        target_bir_lowering=False,
        debug=False,
        num_devices=NCORES,
    )

    xw8 = nc.dram_tensor("xw8", [N8 * P * KW8], F8, kind="ExternalInput")
    xw = nc.dram_tensor("xw", [NF * P * KW], F16, kind="ExternalInput")
    af_d = nc.dram_tensor("af", [P, KT * P], F8, kind="ExternalInput")
    ps_d = nc.dram_tensor("ps", [P, BT * NUM_CLASS], F32, kind="ExternalInput")
    bS_d = nc.dram_tensor("bS", [NUM_CLASS + 1, OUT_L], F32R, kind="ExternalInput")
    out = nc.dram_tensor("out", [B, OUT_L], F16, kind="ExternalOutput")

    with TileContext(nc) as tc:
        with (
            tc.tile_pool(name="xwp", bufs=1) as xwpool,
            tc.tile_pool(name="sml", bufs=1) as spool,
            tc.tile_pool(name="tl", bufs=1) as tpool,
            tc.tile_pool(name="op", bufs=1) as opool,
            tc.tile_pool(name="ps", bufs=1, space="PSUM") as ppool,
        ):
            # ---- resident stream buffers. The fp8 block uses one tile per
            # DoubleRow pair so the PE's dependency is on exactly the two
            # k-tiles it reads (coarser tracking held the first DR main
            # until the whole block landed). ----
            xw8p = [
                xwpool.tile([P, 2 * KW8], F8, name=f"xw8p{p}")
                for p in range(NP8)
            ]
            xw_sb = xwpool.tile([P, NF * KW], F16)
            # one tile PER a-quarter: a single a tile gave the G passes a
            # whole-tile dependency (rearranged APs defeat region tracking),
            # so the first G pass waited for the LAST quarter's DMA — a
            # ~5us PE stall that also reset the p-state ramp
            QW = KT * P // 4  # 1024 fp8 cols per a-quarter (8 k-tiles)
            a_qsb = [
                spool.tile([P, QW], F8, name=f"aq{q}") for q in range(4)
            ]

            dma_done = {}
            # Ring trigger order. fp16 stream first (k8 split across both
            # rings for the earliest PE start, k9 single, then pairs — each
            # ring only runs a few transfers concurrently, so fewer/larger
            # triggers finish the stream tail sooner). The a-quarters and
            # the fp8-pair chunks slot in where the PE's closing fp8 block
            # will need them.
            events = []

            def ev_fp16(k0, n):
                kk = k0 - N8
                src = xw[kk * P * KW : (kk + n) * P * KW].rearrange(
                    "(p d) -> p d", p=P
                )
                dst = xw_sb[:, kk * KW : (kk + n) * KW]

                def go(eng, src=src, dst=dst, k0=k0):
                    dma_done[k0] = eng.dma_start(out=dst, in_=src)
                return go

            def ev_a(q):
                def go(eng, q=q):
                    eng.dma_start(
                        out=a_qsb[q], in_=af_d[:, q * QW : (q + 1) * QW]
                    )
                return go

            def ev_fp8(p):
                src = xw8[2 * p * P * KW8 : (2 * p + 2) * P * KW8].rearrange(
                    "(p d) -> p d", p=P
                )

                def go(eng, src=src, p=p):
                    eng.dma_start(out=xw8p[p], in_=src)
                return go

            # k8 halves on both rings first
            src8 = xw[0 : P * KW].rearrange("(p d) -> p d", p=P)
            nc.sync.dma_start(
                out=xw_sb[:, 0 : KW // 2], in_=src8[:, 0 : KW // 2]
            )
            dma_done[N8] = nc.scalar.dma_start(
                out=xw_sb[:, KW // 2 : KW], in_=src8[:, KW // 2 :]
            )
            # ps early on a ring: the mask chain (DVE reduces -> maskb ->
            # PE transposes) starts from it, and on the gated SWDGE path it
            # crawled through the saturated queues, stalling the PE
            # transposes ~3us
            ps_sb = spool.tile([P, BT * NUM_CLASS], F32)
            nc.sync.dma_start(out=ps_sb, in_=ps_d[:, :])
            # singles k(N8+1..N8+3), then pairs; the LAST pair (k30/31)
            # rides mid-queue — the PE needs it right after k29 (the finale
            # mains run before the fp8 block), and at the back of the queue
            # it landed ~30us, stalling the PE and resetting the p-state
            # for the whole fp8 block. a-quarters and fp8-pair chunks slot
            # in ahead of where the closing fp8 block needs them.
            for k0 in range(N8 + 1, N8 + NSGL):
                events.append(ev_fp16(k0, 1))
            pairs = list(range(N8 + NSGL, KT, 2))
            last = pairs.pop()                  # (30,31)
            mid = len(pairs) // 2
            for i, k0 in enumerate(pairs):
                events.append(ev_fp16(k0, 2))
                if i == mid:
                    events.append(ev_a(1))
                if i == max(len(pairs) - 2, 0):
                    events.append(ev_fp16(last, 2))
            events.append(ev_a(2))
            events.append(ev_a(3))
            for p in range(NP8):
                events.append(ev_fp8(p))
            events.append(ev_a(0))
            for i, go in enumerate(events):
                go(nc.sync if i % 2 == 0 else nc.scalar)

            # ---- bS on the gpsimd SWDGE queue, gated late: it is not
            # needed until the lora tail (~37us) and ungated it competes
            # with the stream for DMA bandwidth ----
            bS_sb = spool.tile([NUM_CLASS + 1, OUT_L], F32R)
            bS_dma = nc.gpsimd.dma_start(out=bS_sb, in_=bS_d[:, :])
            add_dep_helper(
                bS_dma.ins, dma_done[16].ins,
                reason="bS yields early DMA bandwidth to the stream",
            )
            # identity for the PE mask transposes (Pool, dep-free, early)
            ident = spool.tile([P, P], F32)
            make_identity(nc, ident)

            # ---- PE warm-up: ramp the p-state during the DMA lead-in ----
            warm_src = spool.tile([P, P], F16)
            nc.vector.memset(warm_src, 0.0)
            warm_ps = ppool.tile([P, P], F32, tag="warm", name="warm_ps")
            for w in range(WARM):
                nc.tensor.matmul(
                    warm_ps, lhsT=warm_src, rhs=warm_src, start=True, stop=True
                )

            def blk8(p):
                # fp8 stream pair p as [P, 2, KW8]
                return xw8p[p].rearrange("q (two d) -> q two d", two=2)

            def xk(kk):
                return xw_sb[:, kk * KW : kk * KW + B]

            def wk(kk):
                return xw_sb[:, kk * KW + B : kk * KW + B + OUT_L]

            # ---- on-chip fp16 -> fp8 casts of x (k8..31) for the DoubleRow
            # G matmul, all on DVE (~400ns each; Pool casts cost ~1.85us).
            # The mask work is interleaved at the points where its inputs
            # land so nothing queues behind the full cast stream. ----
            x8_sb = spool.tile([P, NF * B], F8)

            def cast_x8(kk):
                nc.vector.tensor_copy(
                    out=x8_sb[:, kk * B : (kk + 1) * B], in_=xk(kk)
                )

            for kk in range(2):
                cast_x8(kk)

            # ---- top-2 threshold per batch row (free-axis reduces: DVE) ----
            m2col = spool.tile([P, BT], F32)
            for bt in range(BT):
                pt = ps_sb[:, bt * NUM_CLASS : (bt + 1) * NUM_CLASS]
                m1 = spool.tile([P, 1], F32, tag=f"m1_{bt}")
                nc.vector.reduce_max(out=m1, in_=pt, axis=X)
                negmask = spool.tile([P, NUM_CLASS], F32, tag=f"nm_{bt}")
                # (pt >= m1) * -1e30  -> additive mask that kills the max
                nc.vector.tensor_scalar(
                    out=negmask,
                    in0=pt,
                    scalar1=m1,
                    scalar2=-1.0e30,
                    op0=mybir.AluOpType.is_ge,
                    op1=mybir.AluOpType.mult,
                )
                p2 = spool.tile([P, NUM_CLASS], F32, tag=f"p2_{bt}")
                nc.vector.tensor_tensor(
                    out=p2, in0=pt, in1=negmask, op=mybir.AluOpType.add
                )
                nc.vector.reduce_max(out=m2col[:, bt : bt + 1], in_=p2, axis=X)

            # mask in [b, c] orientation: maskb[b, c] = ps[b, c] >= m2[b].
            # (The old path transposed m2 via a DRAM bounce + broadcast DMA;
            # those 4-byte-descriptor transfers crawled through the
            # stream-saturated queues and delivered maskT at ~42us,
            # stalling the lora tail ~4us. The PE transposes maskb instead.)
            maskb = tpool.tile([P, BT * NUM_CLASS], F32)
            for bt in range(BT):
                nc.vector.tensor_scalar(
                    out=maskb[:, bt * NUM_CLASS : (bt + 1) * NUM_CLASS],
                    in0=ps_sb[:, bt * NUM_CLASS : (bt + 1) * NUM_CLASS],
                    scalar1=m2col[:, bt : bt + 1],
                    scalar2=None,
                    op0=mybir.AluOpType.is_ge,
                )

            for kk in range(2, NF):
                cast_x8(kk)

            # maskT: PSUM->SBUF copy of the PE-transposed mask; the copy is
            # EMITTED inside the PE loop (after the transposes) so the
            # framework orders it after its writer
            mask_ps = ppool.tile([NUM_CLASS, B], F32, tag="mt", name="mask_ps")
            maskT = tpool.tile([NUM_CLASS, B], F32)
            maskT16 = tpool.tile([NUM_CLASS, B], F32)

            # ht rows: 0..63 filled at the end; row 64 = ones (written early,
            # on Pool: 0*x + 1 from any [1, B] source already in SBUF)
            ht = tpool.tile([NUM_CLASS + 1, B], F32R)
            nc.gpsimd.tensor_scalar(
                out=ht[NUM_CLASS : NUM_CLASS + 1, :],
                in0=xw_sb[0:1, 0:B],
                scalar1=0.0,
                scalar2=1.0,
                op0=mybir.AluOpType.mult,
                op1=mybir.AluOpType.add,
            )

            # ---- PSUM accumulators ----
            mps = [
                ppool.tile([P, OUT_L], F32, tag=f"main{bt}", name=f"main{bt}")
                for bt in range(BT)
            ]
            # G accumulator: partitions 0:64 = sum x8*a8, 64:128 = the
            # a-residual half (both halves of the packed DoubleRow stationary)
            gt_ps = ppool.tile([P, B], F32, tag="gt", name="gt_ps")

            def main_mm8(p, bt):
                b = blk8(p)
                nc.tensor.matmul(
                    mps[bt],
                    lhsT=b[:, :, bt * P : (bt + 1) * P],
                    rhs=b[:, :, B : B + OUT_L],
                    start=False,
                    stop=False,
                    perf_mode=DR,
                )

            def main_mm(kk, bt):
                nc.tensor.matmul(
                    mps[bt],
                    lhsT=xk(kk)[:, bt * P : (bt + 1) * P],
                    rhs=wk(kk),
                    start=(kk == 0),
                    stop=False,
                )

            def g_dr(j, start, stop):
                jj = j % 4
                a_ap = a_qsb[j // 4][:, 2 * jj * P : (2 * jj + 2) * P].rearrange(
                    "q (two m) -> q two m", two=2
                )
                if j < NP8:
                    x_ap = blk8(j)[:, :, 0:B]
                else:
                    kk = 2 * j - N8
                    x_ap = x8_sb[:, kk * B : (kk + 2) * B].rearrange(
                        "q (two n) -> q two n", two=2
                    )
                nc.tensor.matmul(
                    gt_ps,
                    lhsT=a_ap,
                    rhs=x_ap,
                    start=start,
                    stop=stop,
                    perf_mode=DR,
                )

            # ---- PE main stream ----
            # fp16 mains k8..k29 first: by the time the closing fp8 block
            # runs, the clock is fully ramped (an fp8/fp16 alternation or an
            # early idle gap left the DoubleRow passes at the 1.2GHz
            # p-state), the x8 casts and the fp8 stream chunks have long
            # landed, and the mode switch count is down to one in, one out.
            # The 4 mask transposes slot in mid-stream (maskb ready ~15us,
            # and the supply-tight stream gets a breather).
            for kk in range(NF - 2):
                for bt in range(BT):
                    main_mm(kk, bt)
                if kk == 8:
                    for bt in range(BT):
                        nc.tensor.transpose(
                            out=mask_ps[:, bt * P : (bt + 1) * P],
                            in_=maskb[:, bt * NUM_CLASS : (bt + 1) * NUM_CLASS],
                            identity=ident[:, :],
                        )
                    nc.vector.tensor_copy(out=maskT, in_=mask_ps[:, :])
                    # maskT/16 for the a-residual G half, off the tail path
                    nc.vector.tensor_scalar(
                        out=maskT16, in0=maskT, scalar1=1.0 / SR,
                        scalar2=None, op0=mybir.AluOpType.mult,
                    )
            # k30/k31 mains BEFORE the closing fp8 block: PSUM accumulation
            # order is commutative, and this leaves only the 4 lora matmuls
            # + drains after the block (~1.7us less serialized tail)
            for kk in (NF - 2, NF - 1):
                for bt in range(BT):
                    main_mm(kk, bt)
            # closing fp8-DR block: all 16 G pairs (4..15 first — their
            # casts finish earliest; 0..3 last carry the group stop so the
            # DVE lora chain unblocks right here), then the k0-7 DR mains
            # (whose 3.5us cover the DVE/Pool ht chain)
            for j in range(NP8, NPAIR):
                g_dr(j, start=(j == NP8), stop=False)
            for j in range(NP8):
                g_dr(j, start=False, stop=(j == NP8 - 1))
            for p in range(NP8):
                for bt in range(BT):
                    main_mm8(p, bt)

            # ht[0:64] = G8*maskT + Gr*(maskT/16), column-halved. Only DVE
            # (and ACT) may read PSUM, and each op may read at most one PSUM
            # operand: DVE does the four PSUM-read multiplies (half 0 first
            # so the bt0/bt1 lora matmuls unblock early); Pool does the
            # SBUF-only adds.
            t_sb = tpool.tile([NUM_CLASS, B], F32)
            t2_sb = tpool.tile([NUM_CLASS, B], F32)
            for half in (0, 1):
                cs = slice(half * (B // 2), (half + 1) * (B // 2))
                nc.vector.tensor_tensor(
                    out=t_sb[:, cs],
                    in0=gt_ps[0:NUM_CLASS, cs],
                    in1=maskT[:, cs],
                    op=mybir.AluOpType.mult,
                )
                nc.vector.tensor_tensor(
                    out=t2_sb[:, cs],
                    in0=gt_ps[NUM_CLASS : 2 * NUM_CLASS, cs],
                    in1=maskT16[:, cs],
                    op=mybir.AluOpType.mult,
                )
                nc.gpsimd.tensor_tensor(
                    out=ht[0:NUM_CLASS, cs],
                    in0=t_sb[:, cs],
                    in1=t2_sb[:, cs],
                    op=mybir.AluOpType.add,
                )

            # ---- finale: per bt: last two main k, lora tail matmul, scaled
            # copy (divides out the W scale), DMA out — interleaved so each
            # tile's output drain overlaps the remaining compute ----
            o_all = opool.tile([P, BT * OUT_L], F16)
            dma_eng = [nc.sync, nc.scalar, nc.sync, nc.scalar]

            def lora_mm(bt):
                nc.tensor.matmul(
                    mps[bt],
                    lhsT=ht[:, bt * P : (bt + 1) * P],
                    rhs=bS_sb,
                    start=False,
                    stop=True,
                )

            def copy_half(bt, half, eng):
                cs = slice(bt * OUT_L + half * (OUT_L // 2),
                           bt * OUT_L + (half + 1) * (OUT_L // 2))
                ps_cs = slice(half * (OUT_L // 2), (half + 1) * (OUT_L // 2))
                if eng is nc.vector:
                    nc.vector.tensor_scalar(
                        out=o_all[:, cs], in0=mps[bt][:, ps_cs],
                        scalar1=1.0 / SW, scalar2=None,
                        op0=mybir.AluOpType.mult,
                    )
                else:
                    nc.scalar.activation(
                        out=o_all[:, cs], in_=mps[bt][:, ps_cs],
                        func=mybir.ActivationFunctionType.Copy,
                        scale=1.0 / SW,
                    )

            def drain(bt, split=False):
                # scaled cast (divides out the W scale) then DMA out; the
                # last tile splits across DVE+ACT so its pipeline is shorter
                osl = o_all[:, bt * OUT_L : (bt + 1) * OUT_L]
                if split:
                    copy_half(bt, 0, nc.vector)
                    copy_half(bt, 1, nc.scalar)
                    dma_eng[bt].dma_start(
                        out=out[bt * P : (bt + 1) * P, 0 : OUT_L // 2],
                        in_=o_all[:, bt * OUT_L : bt * OUT_L + OUT_L // 2],
                    )
                    dma_eng[bt - 1].dma_start(
                        out=out[bt * P : (bt + 1) * P, OUT_L // 2 :],
                        in_=o_all[:, bt * OUT_L + OUT_L // 2 : (bt + 1) * OUT_L],
                    )
                    return
                copy_half(bt, 0, nc.vector if bt % 2 == 0 else nc.scalar)
                copy_half(bt, 1, nc.vector if bt % 2 == 0 else nc.scalar)
                dma_eng[bt].dma_start(out=out[bt * P : (bt + 1) * P, :], in_=osl)

            # loras as one f32r block after all DR mains (a lora between DR
            # mains both stops its PSUM group too early and re-triggers the
            # mode-alternation clock penalty); each tile's drain overlaps
            # the remaining loras
            for bt in range(BT):
                lora_mm(bt)
                drain(bt)

    nc.finalize()
    _cache[key] = nc
    return nc


def _pack_inputs(x, pseudo_index, weight, bias, lora_A, lora_B):
    """Build the per-core fp8/fp16 streams + replicated small inputs."""
    x16 = x.astype(np.float16)
    xT = np.ascontiguousarray(x16.T)                 # [IN, B] fp16
    x3 = xT.reshape(KT, P, B)
    x3_8 = x3.astype(ml_dtypes.float8_e4m3)          # fp8(fp16(x))

    # fp8 a-stream: per k-tile [a8 (64) | a-residual (64)] columns
    aT = lora_A[:NUM_CLASS].T * SA                   # [IN, 64] scaled
    a8 = aT.astype(ml_dtypes.float8_e4m3)
    ar = ((aT - a8.astype(np.float64)) * SR).astype(ml_dtypes.float8_e4m3)
    a3 = np.concatenate(
        [a8.reshape(KT, P, NUM_CLASS), ar.reshape(KT, P, NUM_CLASS)], axis=2
    )                                                # [KT, P, 128]
    af = np.ascontiguousarray(a3.transpose(1, 0, 2).reshape(P, KT * P))

    ps = np.ascontiguousarray(
        pseudo_index.reshape(BT, P, NUM_CLASS)
        .transpose(1, 0, 2)
        .reshape(P, BT * NUM_CLASS)
    )
    in_maps = []
    for i in range(NCORES):
        o0 = i * OUT_L
        wTs = weight[o0 : o0 + OUT_L].T * SW         # [IN, OUT_L] scaled
        w3_16 = wTs.astype(np.float16).reshape(KT, P, OUT_L)
        # fp8 stream (k0..7): [x8 | w8], packed as per-pair [P, 2*KW8] blocks
        k8b = np.empty((N8, P, KW8), dtype=ml_dtypes.float8_e4m3)
        k8b[:, :, 0:B] = x3_8[:N8]
        k8b[:, :, B:] = wTs.reshape(KT, P, OUT_L)[:N8].astype(
            ml_dtypes.float8_e4m3
        )
        xw8i = np.empty(N8 * P * KW8, dtype=ml_dtypes.float8_e4m3)
        for p in range(NP8):
            xw8i[2 * p * P * KW8 : (2 * p + 2) * P * KW8] = (
                k8b[2 * p : 2 * p + 2].transpose(1, 0, 2).reshape(-1)
            )
        # fp16 stream (k8..31): [x16 | w16], packed per CHUNKS16 blocks
        k3 = np.empty((NF, P, KW), dtype=np.float16)
        k3[:, :, 0:B] = x3[N8:]
        k3[:, :, B:] = w3_16[N8:]
        xwi = np.empty(NF * P * KW, dtype=np.float16)
        for kk0, n in CHUNKS16:
            xwi[kk0 * P * KW : (kk0 + n) * P * KW] = (
                k3[kk0 : kk0 + n].transpose(1, 0, 2).reshape(-1)
            )
        bS = np.empty((NUM_CLASS + 1, OUT_L), dtype=np.float32)
        # lora scaling 16 * (SW / SA) = 16; double bias * SW = 512
        bS[:NUM_CLASS] = (16.0 * SW / SA) * lora_B[o0 : o0 + OUT_L, :NUM_CLASS].T
        bS[NUM_CLASS] = 2.0 * SW * bias[o0 : o0 + OUT_L]
        in_maps.append({
            "xw8": xw8i, "xw": xwi,
            "af": af, "ps": ps, "bS": bS,
        })
    return in_maps


def kernel(x, pseudo_index, weight, bias, lora_A, lora_B):
    global last_results
    x = np.ascontiguousarray(np.asarray(x, dtype=np.float32))
    pseudo_index = np.ascontiguousarray(np.asarray(pseudo_index, dtype=np.float32))
    weight = np.asarray(weight, dtype=np.float32)
    bias = np.asarray(bias, dtype=np.float32)
    lora_A = np.asarray(lora_A, dtype=np.float32)
    lora_B = np.asarray(lora_B, dtype=np.float32)

    nc = _build()
    in_maps = _pack_inputs(x, pseudo_index, weight, bias, lora_A, lora_B)
    res = run_bass_kernel_spmd(nc, in_maps, list(range(NCORES)))
    last_results = res
    return np.hstack(
        [res.results[i]["out"] for i in range(NCORES)]
    ).astype(np.float32)
